# revision 23
# baseline (speedup 1.0000x reference)
"""CrossAttention Trainium2 kernel (v3).

Full inputs -> shard over 8 cores (batch x head-group) -> Bass kernel ->
host gather (sum head-group partials per batch + bias).

Per-core layout (B=2 batches x 4 head-groups of 4 heads):
  xT    [1024, 2048]  x[b].T
  ctxT  [1024, 2048]  context[b].T
  wqT   [1024, 256]   Wq[rows(g)].T      (rows(g) = g*256 : (g+1)*256)
  wkT   [1024, 256]
  wvT   [1024, 256]
  woT   [256, 1024]   Wo[:, rows(g)].T
  out y [2048, 1024]  partial (sum over g gives batch output; bias on host)

The kernel is ACT(exp)-bound (~147us of exp on the scalar engine), so the
schedule keeps ACT streaming with no boundary stalls:
  - x/ctx staged in SBUF up front; DMAs ordered so k-proj starts early
  - dense prologue: k/v projections + q-proj(0)
  - per (n, hp): 8 chunk-pair iterations: row-packed score MMs into
    [128,1024] psum tiles (2-buf pool) -> one 1024-wide exp ACTIVATE per
    head -> AV matmuls (V carries a ones column so softmax denominators
    fall out of the same matmul into U row 64)
  - every serial tail is interleaved into the NEXT m-loop's slack so the
    tensor queue never head-of-line-blocks the activations:
      * tail(n,hp0) (recip chain + E-matmul + A mul) -> spread over
        m-loop(n,hp1) iterations 0..3
      * q-proj(n+1) -> spread over m-loop(n,hp1) iterations 0..7
      * tail(n,hp1) + out-proj(n) -> spread over m-loop(n+1,hp0)
"""
import numpy as np
import ml_dtypes

HEADS = 16
DIM_HEAD = 64
D_MODEL = 1024
N_CORES = 8


def build_nc(n_q=2048, n_kv=2048, d_model=1024, n_heads=4, d_head=64, nt=512):
    """Build the per-core Bass module."""
    import concourse.bass as bass
    import concourse.mybir as mybir
    import concourse.tile as tile
    from concourse import bacc

    FP32 = mybir.dt.float32
    BF16 = mybir.dt.bfloat16
    EXP = mybir.ActivationFunctionType.Exp
    P = 128

    inner = n_heads * d_head          # 256
    ND = d_model // P                 # 8 contraction chunks
    NI = inner // P                   # 2 head-pairs
    NNT = n_q // nt                   # 4 query tiles
    NMT = n_kv // nt                  # 4 key tiles
    NMC = n_kv // P                   # 16 key chunks
    NJ = d_model // nt                # 2 output col tiles
    NHP = n_heads // 2                # 2 head pairs
    MPAIRS = NMC // 2                 # 8 chunk pairs

    nc = bacc.Bacc(None, target_bir_lowering=False, debug=False)

    xT = nc.dram_tensor("xT", [d_model, n_q], BF16, kind="ExternalInput")
    ctxT = nc.dram_tensor("ctxT", [d_model, n_kv], BF16, kind="ExternalInput")
    wqT = nc.dram_tensor("wqT", [d_model, inner], BF16, kind="ExternalInput")
    wkT = nc.dram_tensor("wkT", [d_model, inner], BF16, kind="ExternalInput")
    wvT = nc.dram_tensor("wvT", [d_model, inner], BF16, kind="ExternalInput")
    woT = nc.dram_tensor("woT", [inner, d_model], BF16, kind="ExternalInput")
    y = nc.dram_tensor("y", [n_q, d_model], FP32, kind="ExternalOutput")

    xT_r = xT.ap().rearrange("(c p) n -> p c n", p=P)      # [128, ND, n_q]
    ctxT_r = ctxT.ap().rearrange("(c p) m -> p c m", p=P)
    wqT_r = wqT.ap().rearrange("(c p) i -> p c i", p=P)
    wkT_r = wkT.ap().rearrange("(c p) i -> p c i", p=P)
    wvT_r = wvT.ap().rearrange("(c p) i -> p c i", p=P)
    woT_r = woT.ap().rearrange("(c p) j -> p c j", p=P)    # [128, NI, d_model]

    scale = float(d_head) ** -0.5

    with tile.TileContext(nc) as tc:
        with (
            tc.tile_pool(name="persist", bufs=1) as persist,
            tc.tile_pool(name="vpool", bufs=NMC) as vpool,
        ):
            # ---------------- persistent tiles ----------------
            qT_sb = [persist.tile([P, n_q], BF16, tag=f"qT{i}", name=f"qT{i}")
                     for i in range(NI)]
            kT_sb = [persist.tile([P, n_kv], BF16, tag=f"kT{i}", name=f"kT{i}")
                     for i in range(NI)]
            woT_sb = persist.tile([P, NI, d_model], BF16, tag="woT")
            wq_sb = persist.tile([P, ND, inner], BF16, tag="wq")
            wk_sb = persist.tile([P, ND, inner], BF16, tag="wk")
            wv_sb = persist.tile([P, ND, inner], BF16, tag="wv")
            xq_sb = persist.tile([P, ND, n_q], BF16, tag="xq")
            ck_sb = persist.tile([P, ND, n_kv], BF16, tag="ck")

            # warm the ACT exp table before any real work
            warm = persist.tile([P, 8], FP32, tag="warm")
            nc.vector.memset(warm[:], 0.0)
            nc.scalar.activation(warm[0:1, 0:8], warm[0:1, 0:8], EXP)

            # E: broadcast matrix (row0 -> out rows 0..63, row64 -> 64..127)
            scratch = persist.tile([P, nt], FP32, tag="scratch")
            nc.vector.memset(scratch[:], 0.0)
            E_sb = persist.tile([P, P], FP32, tag="E")
            nc.vector.tensor_copy(E_sb[:], scratch[:, 0:P])
            ones_sc = persist.tile([P, 64], FP32, tag="ones_sc")
            nc.vector.memset(ones_sc[:], 1.0)
            nc.vector.tensor_copy(E_sb[0:1, 0:64], ones_sc[0:1, :])
            nc.vector.tensor_copy(E_sb[64:65, 64:128], ones_sc[64:65, :])
            # ssb rows 1..63 stay 1.0 so the [0:65] reciprocal is finite
            ssb_p = persist.tile([P, nt], FP32, tag="ssb_p")
            nc.vector.memset(ssb_p[:], 1.0)
            # rs32 rows 65..127 stay 0 (E-matmul reads all partitions)
            rs32 = persist.tile([P, nt], FP32, tag="rs32")
            nc.vector.memset(rs32[:], 0.0)
            # v tiles: per m-chunk [128, heads, 65]; col 64 is the ones
            # column (softmax denominator trick)
            v_sb = [vpool.tile([P, n_heads, 65], BF16, tag="vsb", name=f"vsb{m}")
                    for m in range(NMC)]
            for m in range(NMC):
                nc.vector.memset(v_sb[m][:, :, 64:65], 1.0)

            # ------------- input DMAs (k-proj critical path first) ------
            nc.sync.dma_start(wk_sb[:], wkT_r[:, :, :])
            nc.sync.dma_start(ck_sb[:, :, 0:nt], ctxT_r[:, :, 0:nt])
            nc.sync.dma_start(ck_sb[:, :, nt:2 * nt], ctxT_r[:, :, nt:2 * nt])
            nc.sync.dma_start(wq_sb[:], wqT_r[:, :, :])
            nc.sync.dma_start(xq_sb[:, :, 0:nt], xT_r[:, :, 0:nt])
            nc.sync.dma_start(wv_sb[:], wvT_r[:, :, :])
            for m in range(2, NMT):
                msl = slice(m * nt, (m + 1) * nt)
                nc.sync.dma_start(ck_sb[:, :, msl], ctxT_r[:, :, msl])
            nc.sync.dma_start(woT_sb[:], woT_r[:, :, :])
            for n in range(1, NNT):
                nsl = slice(n * nt, (n + 1) * nt)
                nc.sync.dma_start(xq_sb[:, :, nsl], xT_r[:, :, nsl])

            # ------- stage 1 prologue: k-proj (head-pair 0) + q(0) ------
            # v-proj and k-proj(i=1) are folded into m-loop(0,hp0) below.
            with tc.tile_pool(name="s1ps", bufs=4, space="PSUM") as s1ps:
                for m in range(1):
                    msl = slice(m * nt, (m + 1) * nt)
                    ps = s1ps.tile([P, nt], FP32, tag="qk")
                    for d in range(ND):
                        nc.tensor.matmul(
                            ps[:],
                            wk_sb[:, d, 0:P],
                            ck_sb[:, d, msl],
                            start=(d == 0), stop=(d == ND - 1))
                    nc.vector.tensor_copy(kT_sb[0][:, msl], ps[:])
                for i in range(NI):
                    ps = s1ps.tile([P, nt], FP32, tag="qk")
                    for d in range(ND):
                        nc.tensor.matmul(
                            ps[:],
                            wq_sb[:, d, i * P:(i + 1) * P],
                            xq_sb[:, d, 0:nt],
                            start=(d == 0), stop=(d == ND - 1))
                    nc.vector.tensor_copy(qT_sb[i][:, 0:nt], ps[:])

            # ---------------- stages 2-4 ----------------
            with (
                tc.tile_pool(name="psq", bufs=2, space="PSUM") as psq,
                tc.tile_pool(name="upool", bufs=2, space="PSUM") as upool,
                tc.tile_pool(name="mpool", bufs=2, space="PSUM") as mpool,
                tc.tile_pool(name="expp", bufs=6) as expp,
                tc.tile_pool(name="s2sb", bufs=3) as s2sb,
                tc.tile_pool(name="apool", bufs=4) as apool,
                tc.tile_pool(name="ypool", bufs=6) as ypool,
            ):
                A_tiles = {}
                U_tiles = {}
                qp_tiles = {}

                def tail_dve(n, hp):
                    """Evacuate U, gather denominators, reciprocal (DVE)."""
                    U0, U1 = U_tiles[(n, hp)]
                    usb = s2sb.tile([P, nt], BF16, tag="usb",
                                    name=f"usb{n}_{hp}")
                    nc.vector.tensor_copy(usb[0:64, :], U0[0:64, :])
                    nc.vector.tensor_copy(usb[64:128, :], U1[0:64, :])
                    nc.vector.tensor_copy(ssb_p[0:1, :], U0[64:65, :])
                    nc.vector.tensor_copy(ssb_p[64:65, :], U1[64:65, :])
                    with nc.allow_low_precision(reason="softmax recip"):
                        nc.vector.reciprocal(rs32[0:65, :], ssb_p[0:65, :])
                    U_tiles[(n, hp)] = (usb,)

                def tail_emm(n, hp):
                    bps = mpool.tile([P, nt], FP32, tag="m",
                                     name=f"bps{n}_{hp}")
                    nc.tensor.matmul(bps[:], E_sb[:], rs32[:],
                                     start=True, stop=True)
                    U_tiles[(n, hp)] = U_tiles[(n, hp)] + (bps,)

                def tail_amul(n, hp):
                    usb, bps = U_tiles.pop((n, hp))
                    A = apool.tile([P, nt], BF16, tag="A", name=f"A{n}_{hp}")
                    A_tiles[(n, hp)] = A
                    nc.vector.tensor_mul(A[:], usb[:], bps[:])

                def qproj_piece(n, i, ph):
                    """Two accumulating MMs of q-proj chain i; cast at ph 3."""
                    nsl = slice(n * nt, (n + 1) * nt)
                    if ph == 0:
                        qp_tiles[(n, i)] = mpool.tile(
                            [P, nt], FP32, tag="m", name=f"qp{n}_{i}")
                    ps = qp_tiles[(n, i)]
                    for d in range(2 * ph, 2 * ph + 2):
                        nc.tensor.matmul(
                            ps[:],
                            wq_sb[:, d, i * P:(i + 1) * P],
                            xq_sb[:, d, nsl],
                            start=(d == 0), stop=(d == ND - 1))
                    if ph == 3:
                        nc.vector.tensor_copy(qT_sb[i][:, nsl], ps[:])
                        del qp_tiles[(n, i)]

                def vproj_chunk(m):
                    psv = mpool.tile([P, n_heads, d_head], FP32, tag="m",
                                     name=f"vp{m}")
                    for d in range(ND):
                        nc.tensor.matmul(
                            psv[:], ck_sb[:, d, m * P:(m + 1) * P],
                            wv_sb[:, d, :],
                            start=(d == 0), stop=(d == ND - 1))
                    nc.vector.tensor_copy(v_sb[m][:, :, 0:64], psv[:])

                def kproj_tile(i, m):
                    msl = slice(m * nt, (m + 1) * nt)
                    ps = mpool.tile([P, nt], FP32, tag="m", name=f"kp{i}_{m}")
                    for d in range(ND):
                        nc.tensor.matmul(
                            ps[:],
                            wk_sb[:, d, i * P:(i + 1) * P],
                            ck_sb[:, d, msl],
                            start=(d == 0), stop=(d == ND - 1))
                    nc.vector.tensor_copy(kT_sb[i][:, msl], ps[:])

                def outproj_group(n, g):
                    q, j = g // NJ, g % NJ
                    qsl = slice(q * P, (q + 1) * P)
                    jsl = slice(j * nt, (j + 1) * nt)
                    yps = mpool.tile([P, nt], FP32, tag="m",
                                     name=f"yp{n}_{q}_{j}")
                    for c in range(NI):
                        nc.tensor.matmul(
                            yps[:], A_tiles[(n, c)][:, qsl],
                            woT_sb[:, c, jsl],
                            start=(c == 0), stop=(c == NI - 1))
                    ysb = ypool.tile([P, nt], FP32, tag="ysb")
                    nc.vector.tensor_copy(ysb[:], yps[:])
                    nc.sync.dma_start(
                        y.ap()[n * nt + q * P:n * nt + (q + 1) * P, jsl],
                        ysb[:])

                prefetched = {}

                def emit_scores(n, hp, mp):
                    """Score pair matmuls + exp ACTIVATEs for one mp."""
                    nsl = slice(n * nt, (n + 1) * nt)
                    pq0 = psq.tile([P, 2 * nt], FP32, tag="psq")
                    pq1 = psq.tile([P, 2 * nt], FP32, tag="psq")
                    for s in range(2):
                        m = 2 * mp + s
                        msl = slice(m * P, (m + 1) * P)
                        ssl = slice(s * nt, (s + 1) * nt)
                        nc.tensor.matmul(
                            pq0[:, ssl],
                            kT_sb[hp][0:64, msl],
                            qT_sb[hp][0:64, nsl],
                            start=True, stop=True, tile_position=(0, 0))
                        nc.tensor.matmul(
                            pq1[:, ssl],
                            kT_sb[hp][64:128, msl],
                            qT_sb[hp][64:128, nsl],
                            start=True, stop=True, tile_position=(64, 0))
                    ex0 = expp.tile([P, 2 * nt], BF16, tag="ex")
                    ex1 = expp.tile([P, 2 * nt], BF16, tag="ex")
                    nc.scalar.activation(ex0[:], pq0[:], EXP, scale=scale)
                    nc.scalar.activation(ex1[:], pq1[:], EXP, scale=scale)
                    return ex0, ex1

                def mloop(n, hp, extras, nxt=None):
                    h0, h1 = 2 * hp, 2 * hp + 1
                    U0 = upool.tile([P, nt], FP32, tag="U", name=f"U0_{n}_{hp}")
                    U1 = upool.tile([P, nt], FP32, tag="U", name=f"U1_{n}_{hp}")
                    U_tiles[(n, hp)] = (U0, U1)
                    for mp in range(MPAIRS):
                        if mp == 0 and (n, hp) in prefetched:
                            ex0, ex1 = prefetched.pop((n, hp))
                        else:
                            ex0, ex1 = emit_scores(n, hp, mp)
                        # extras run on the tensor queue while the AV
                        # matmuls below would be waiting on the activations
                        for fn in extras.get(mp, []):
                            fn()
                        if mp == MPAIRS - 1 and nxt is not None:
                            # prefetch the next loop's first score pair so
                            # ACT rolls straight across the loop boundary
                            prefetched[nxt] = emit_scores(nxt[0], nxt[1], 0)
                        for s in range(2):
                            m = 2 * mp + s
                            ssl = slice(s * nt, (s + 1) * nt)
                            first = (mp == 0 and s == 0)
                            last = (mp == MPAIRS - 1 and s == 1)
                            nc.tensor.matmul(
                                U0[0:65, :], v_sb[m][:, h0, :], ex0[:, ssl],
                                start=first, stop=last)
                            nc.tensor.matmul(
                                U1[0:65, :], v_sb[m][:, h1, :], ex1[:, ssl],
                                start=first, stop=last)

                for n in range(NNT):
                    # hp0 m-loop: absorb tail(n-1,hp1) + out-proj(n-1)
                    ex0_sched = {}
                    if n == 0:
                        # fold v-proj and the remaining k-proj tiles into
                        # the first m-loop's AV-wait slack; k-proj(i, m) is
                        # always emitted before the scores that consume it
                        kp_sched = {0: (0, 1), 1: (1, 0), 2: (0, 2),
                                    3: (1, 1), 4: (0, 3), 5: (1, 2),
                                    7: (1, 3)}
                        for mp in range(MPAIRS):
                            ent = [lambda mp=mp: vproj_chunk(2 * mp),
                                   lambda mp=mp: vproj_chunk(2 * mp + 1)]
                            if mp in kp_sched:
                                i_, m_ = kp_sched[mp]
                                ent.append(
                                    lambda i_=i_, m_=m_: kproj_tile(i_, m_))
                            ex0_sched[mp] = ent
                    if n > 0:
                        ex0_sched = {
                            0: [lambda: tail_dve(n - 1, 1)],
                            2: [lambda: tail_emm(n - 1, 1)],
                            3: [lambda: tail_amul(n - 1, 1),
                                lambda: outproj_group(n - 1, 0)],
                            4: [lambda: outproj_group(n - 1, 1),
                                lambda: outproj_group(n - 1, 2)],
                            5: [lambda: outproj_group(n - 1, 3),
                                lambda: outproj_group(n - 1, 4)],
                            6: [lambda: outproj_group(n - 1, 5),
                                lambda: outproj_group(n - 1, 6)],
                            7: [lambda: outproj_group(n - 1, 7)],
                        }
                    mloop(n, 0, ex0_sched, nxt=(n, 1))
                    # hp1 m-loop: absorb tail(n,hp0) + q-proj(n+1)
                    ex1_sched = {
                        0: [lambda: tail_dve(n, 0)],
                        2: [lambda: tail_emm(n, 0)],
                        3: [lambda: tail_amul(n, 0)],
                    }
                    if n + 1 < NNT:
                        for ph in range(4):
                            ex1_sched.setdefault(ph, []).append(
                                lambda ph=ph: qproj_piece(n + 1, 0, ph))
                            ex1_sched.setdefault(4 + ph, []).append(
                                lambda ph=ph: qproj_piece(n + 1, 1, ph))
                    mloop(n, 1, ex1_sched,
                          nxt=(n + 1, 0) if n + 1 < NNT else None)
                # final tail + out-proj, serial
                tail_dve(NNT - 1, 1)
                tail_emm(NNT - 1, 1)
                tail_amul(NNT - 1, 1)
                for g in range(8):
                    outproj_group(NNT - 1, g)

    nc.compile()
    return nc


def shard_inputs(x, context, Wq, Wk, Wv, Wo):
    """Per-core input dicts: core c -> (batch c//4, head-group c%4)."""
    in_maps = []
    for c in range(N_CORES):
        b, g = c // 4, c % 4
        rows = slice(g * 256, (g + 1) * 256)
        bf = ml_dtypes.bfloat16
        in_maps.append({
            "xT": np.ascontiguousarray(x[b].T).astype(bf),
            "ctxT": np.ascontiguousarray(context[b].T).astype(bf),
            "wqT": np.ascontiguousarray(Wq[rows].T).astype(bf),
            "wkT": np.ascontiguousarray(Wk[rows].T).astype(bf),
            "wvT": np.ascontiguousarray(Wv[rows].T).astype(bf),
            "woT": np.ascontiguousarray(Wo[:, rows].T).astype(bf),
        })
    return in_maps


_CACHE = {}


def _get_nc():
    if "nc" not in _CACHE:
        _CACHE["nc"] = build_nc()
    return _CACHE["nc"]


def kernel(x, context, Wq, Wk, Wv, Wo, bo, _trace=False):
    from concourse.bass_utils import run_bass_kernel_spmd

    x = np.asarray(x, dtype=np.float32)
    context = np.asarray(context, dtype=np.float32)
    in_maps = shard_inputs(x, context,
                           np.asarray(Wq, np.float32), np.asarray(Wk, np.float32),
                           np.asarray(Wv, np.float32), np.asarray(Wo, np.float32))
    nc = _get_nc()
    res = run_bass_kernel_spmd(nc, in_maps, core_ids=list(range(N_CORES)),
                               trace=_trace)
    B, N, _ = x.shape
    out = np.zeros((B, N, D_MODEL), dtype=np.float32)
    for c in range(N_CORES):
        out[c // 4] += res.results[c]["y"]
    out += np.asarray(bo, np.float32)[None, None, :]
    if _trace:
        _CACHE["last_results"] = res
    return out
